# revision 10
# baseline (speedup 1.0000x reference)
"""Trainium2 Bass kernel for the contrastive memory-bank loss.

Strategy: data-parallel over pixels. Host-side we drop masked-out pixels
(they contribute nothing), pad to a multiple of 8*128, and shard the
surviving pixels across 8 cores. The small memory bank is replicated.

Per-pixel math (temp=0.5, S=256, eps=1e-12), for pixel p with label i,
half h = 1-wm, D = total - block_sum[i] + eps:
    term_sum(p) = sum_s log(E_s + D) - sum_s log(E_s)
with E_s = exp(cos_s/temp) over the selected half of class i.
Since D ~ 9e3 >> E_s ~ 1, log(E_s + D) = log(D) + E_s/D - O((E_s/D)^2),
so  term_sum = S*log(D) + (sum_s E_s)/D - (sum_s cos_s)/temp
to relative accuracy ~1e-9.  Only per-(class,half) sums of E and of cos
are needed - no per-element logs over the big [P, C*2S] matrix.

Each core returns per-class partial sums (contrib, count); the host
all-reduces the 8 partials and applies the final scalar normalization.
"""

import sys

sys.path.insert(0, "/opt/trn_rl_repo")

import numpy as np
import ml_dtypes

import concourse.bass as bass
import concourse.bacc as bacc
import concourse.tile as tile
from concourse import mybir
from concourse.bass_utils import run_bass_kernel_spmd

F = 256          # feature dim
C = 19           # num classes
S = 256          # half-bank size
TWO_S = 2 * S
M = C * TWO_S    # 9728 memory entries
J = 2 * C        # 38 (class, half) blocks
N_CORES = 8
TEMP = 0.5
EPS = 1e-12

f32 = mybir.dt.float32
bf16 = mybir.dt.bfloat16
AF = mybir.ActivationFunctionType
ALU = mybir.AluOpType
X = mybir.AxisListType.X


def build(P):
    """Build the per-core Bass program for P pixels per core (P % 128 == 0)."""
    T = P // 128
    nc = bacc.Bacc("TRN2", target_bir_lowering=False, debug=False,
                   num_devices=N_CORES)

    feats_d = nc.dram_tensor("feats", [F, P], f32, kind="ExternalInput")
    memT_d = nc.dram_tensor("memT", [F, M], bf16, kind="ExternalInput")
    labf_d = nc.dram_tensor("labf", [128, T], f32, kind="ExternalInput")
    jself_d = nc.dram_tensor("jself", [128, T], f32, kind="ExternalInput")
    mskf_d = nc.dram_tensor("mskf", [128, T], f32, kind="ExternalInput")
    out_d = nc.dram_tensor("out", [2, C], f32, kind="ExternalOutput")

    with tile.TileContext(nc) as tc:
        with (
            tc.tile_pool(name="const", bufs=1) as const,
            tc.tile_pool(name="persist", bufs=1) as persist,
            tc.tile_pool(name="mem", bufs=1) as mem,
            tc.tile_pool(name="work", bufs=3) as work,
            tc.tile_pool(name="epool", bufs=2) as epool,
        ):
            # ---- constants ----
            iota_i = const.tile([128, J], mybir.dt.int32, tag="iotai")
            nc.gpsimd.iota(iota_i, pattern=[[1, J]], base=0,
                           channel_multiplier=0)
            iota38 = const.tile([128, J], f32, tag="iota38")
            nc.vector.tensor_copy(out=iota38, in_=iota_i)
            ones_col = const.tile([128, 1], f32, tag="ones_col")
            nc.vector.memset(ones_col, 1.0)
            ones_row = const.tile([1, 128], bf16, tag="ones_row")
            nc.vector.memset(ones_row, 1.0)

            # ---- small per-pixel inputs ----
            labf = persist.tile([128, T], f32, tag="labf")
            nc.sync.dma_start(out=labf, in_=labf_d[:, :])
            jself = persist.tile([128, T], f32, tag="jself")
            nc.sync.dma_start(out=jself, in_=jself_d[:, :])
            mskf = persist.tile([128, T], f32, tag="mskf")
            nc.sync.dma_start(out=mskf, in_=mskf_d[:, :])

            # long-lived big tensors
            fb16 = [persist.tile([128, P], bf16, tag=f"fb{k}", name=f"fb{k}")
                    for k in range(2)]
            mn_k = [mem.tile([128, M], bf16, tag=f"mn{k}", name=f"mn{k}")
                    for k in range(2)]
            s_tiles = persist.tile([128, T], f32, tag="stl")
            hcos = persist.tile([128, T * J], f32, tag="hcos")

            # ================= PREP (scoped; freed before main) ========
            with (
                tc.tile_pool(name="prep", bufs=2) as prep,
                tc.tile_pool(name="mraw_p", bufs=1) as mraw_p,
                tc.tile_pool(name="rows", bufs=1) as rows,
                tc.tile_pool(name="dram", bufs=2, space="DRAM") as dram,
            ):
                # ---- feats: load, bf16 cast, per-pixel 1/(fn*temp) ----
                fsq = []
                for k in range(2):
                    fk = prep.tile([128, P], f32, tag=f"f{k}", bufs=1)
                    nc.sync.dma_start(out=fk,
                                      in_=feats_d[k * 128:(k + 1) * 128, :])
                    nc.vector.tensor_copy(out=fb16[k], in_=fk)
                    sq = prep.tile([128, P], f32, tag=f"fsq{k}", bufs=1)
                    nc.scalar.activation(out=sq, in_=fk, func=AF.Square)
                    fsq.append(sq)
                with tc.tile_pool(name="psum_fn", bufs=1,
                                  space="PSUM") as psum_fn_p:
                    psum_fn = psum_fn_p.tile([1, P], f32, tag="fn")
                    for off in range(0, P, 512):
                        n = min(512, P - off)
                        for k in range(2):
                            nc.tensor.matmul(psum_fn[:, off:off + n],
                                             ones_col,
                                             fsq[k][:, off:off + n],
                                             start=(k == 0), stop=(k == 1))
                    fn_row = prep.tile([1, P], f32, tag="fnrow", bufs=1)
                    nc.vector.tensor_copy(out=fn_row, in_=psum_fn)
                d_fn = dram.tile([1, P], f32, tag="dfn")
                nc.sync.dma_start(out=d_fn, in_=fn_row[0:1, :])
                s_pre = persist.tile([128, T], f32, tag="spre")
                nc.sync.dma_start(
                    out=s_pre, in_=d_fn.rearrange("o (t p) -> (o p) t", p=128))
                r1 = prep.tile([128, T], f32, tag="r1")
                nc.vector.reciprocal(out=r1, in_=s_pre)
                # sqrt((1/temp^2) * (1/fn^2)) = 1/(fn*temp)
                nc.scalar.activation(out=s_tiles, in_=r1, func=AF.Sqrt,
                                     scale=1.0 / (TEMP * TEMP))

                # ---- memory bank: load, norms, normalize (bf16) ----
                mraw = []
                for k in range(2):
                    mr = mraw_p.tile([128, M], bf16, tag=f"mraw{k}")
                    nc.sync.dma_start(out=mr,
                                      in_=memT_d[k * 128:(k + 1) * 128, :])
                    mraw.append(mr)
                rinv_t = prep.tile([128, M // 128], f32, tag="rinvt")
                row_b = rows.tile([1, M], bf16, tag="rowb")
                with tc.tile_pool(name="psum_mn", bufs=2,
                                  space="PSUM") as psum_mn_p:
                    for ci in range(C):
                        sl = slice(ci * 512, ci * 512 + 512)
                        pmn = psum_mn_p.tile([1, 512], f32, tag="mn")
                        for k in range(2):
                            sqm = prep.tile([128, 512], f32, tag="sqm")
                            nc.scalar.activation(out=sqm, in_=mraw[k][:, sl],
                                                 func=AF.Square)
                            nc.tensor.matmul(pmn, ones_col, sqm,
                                             start=(k == 0), stop=(k == 1))
                        stg = prep.tile([1, 512], f32, tag="stg")
                        nc.vector.tensor_copy(out=stg, in_=pmn)
                        d_ch = dram.tile([1, 512], f32, tag="dch")
                        nc.sync.dma_start(out=d_ch, in_=stg[0:1, :])
                        nc.sync.dma_start(
                            out=rinv_t[:, ci * 4:(ci + 1) * 4],
                            in_=d_ch.rearrange("o (q p) -> (o p) q", p=128))
                nc.vector.reciprocal(out=rinv_t, in_=rinv_t)
                nc.scalar.activation(out=rinv_t, in_=rinv_t, func=AF.Sqrt)
                rinv_tb = prep.tile([128, M // 128], bf16, tag="rinvtb")
                nc.vector.tensor_copy(out=rinv_tb, in_=rinv_t)
                d_rv = dram.tile([1, M], bf16, tag="drv")
                nc.sync.dma_start(
                    out=d_rv.rearrange("o (c p) -> (o p) c", p=128),
                    in_=rinv_tb)
                nc.sync.dma_start(out=row_b[0:1, :], in_=d_rv)
                with tc.tile_pool(name="psum_bc", bufs=2,
                                  space="PSUM") as psum_bc_p:
                    for ci in range(C):
                        sl = slice(ci * 512, ci * 512 + 512)
                        pbc = psum_bc_p.tile([128, 512], f32, tag="bc")
                        nc.tensor.matmul(pbc, ones_row, row_b[:, sl],
                                         start=True, stop=True)
                        for k in range(2):
                            nc.vector.tensor_mul(out=mn_k[k][:, sl],
                                                 in0=mraw[k][:, sl], in1=pbc)

                # ---- per-(class,half) mem sums -> cos sums, all tiles ----
                hv = []
                for k in range(2):
                    hvf = prep.tile([128, J], f32, tag=f"hvf{k}")
                    nc.vector.tensor_reduce(
                        out=hvf,
                        in_=mn_k[k].rearrange("p (j s) -> p j s", s=S),
                        axis=X, op=ALU.add)
                    hvb = prep.tile([128, J], bf16, tag=f"hv{k}")
                    nc.vector.tensor_copy(out=hvb, in_=hvf)
                    hv.append(hvb)
                with tc.tile_pool(name="psum_hc", bufs=2,
                                  space="PSUM") as psum_hc_p:
                    for t in range(T):
                        phc = psum_hc_p.tile([128, J], f32, tag="hc")
                        for k in range(2):
                            nc.tensor.matmul(
                                phc, fb16[k][:, t * 128:(t + 1) * 128],
                                hv[k], start=(k == 0), stop=(k == 1))
                        nc.scalar.copy(out=hcos[:, t * J:(t + 1) * J],
                                       in_=phc)
            # ================= end PREP ================================

            # ---- accumulators ----
            acc_c = persist.tile([128, C], f32, tag="acc_c")
            nc.vector.memset(acc_c, 0.0)
            acc_n = persist.tile([128, C], f32, tag="acc_n")
            nc.vector.memset(acc_n, 0.0)

            # ---- main loop over pixel tiles ----
            batches = [(0, 4), (4, 4), (8, 4), (12, 4), (16, 3)]
            with tc.tile_pool(name="psum_mm", bufs=2, space="PSUM") as psum_mm:
                for t in range(T):
                    ts = slice(t * 128, (t + 1) * 128)
                    s_col = s_tiles[:, t:t + 1]
                    E = epool.tile([128, J, S], bf16, tag="E")
                    for c0, nb in batches:
                        ps = psum_mm.tile([128, 4 * 512], f32, tag="mm")
                        for k in range(2):
                            for i in range(nb):
                                c = c0 + i
                                nc.tensor.matmul(
                                    ps[:, i * 512:(i + 1) * 512],
                                    fb16[k][:, ts],
                                    mn_k[k][:, c * 512:(c + 1) * 512],
                                    start=(k == 0), stop=(k == 1))
                        nc.scalar.activation(
                            out=E[:, 2 * c0:2 * (c0 + nb), :],
                            in_=ps[:, :nb * 512], func=AF.Exp, scale=s_col)
                    hsum = work.tile([128, J], f32, tag="hsum")
                    nc.vector.tensor_reduce(out=hsum, in_=E, axis=X,
                                            op=ALU.add)
                    h3 = hsum.rearrange("p (c h) -> p c h", h=2)
                    bsum = work.tile([128, C], f32, tag="bsum")
                    nc.vector.tensor_add(out=bsum, in0=h3[:, :, 0],
                                         in1=h3[:, :, 1])
                    total = work.tile([128, 1], f32, tag="total")
                    nc.vector.tensor_reduce(out=total, in_=bsum, axis=X,
                                            op=ALU.add)
                    j19 = work.tile([128, C], f32, tag="j19")
                    ownb = work.tile([128, 1], f32, tag="ownb")
                    nc.vector.scalar_tensor_tensor(
                        out=j19, in0=iota38[:, :C], scalar=labf[:, t:t + 1],
                        in1=bsum, op0=ALU.is_equal, op1=ALU.mult,
                        accum_out=ownb)
                    D = work.tile([128, 1], f32, tag="D")
                    nc.vector.scalar_tensor_tensor(
                        out=D, in0=total, scalar=float(EPS), in1=ownb,
                        op0=ALU.add, op1=ALU.subtract)
                    j38 = work.tile([128, J], f32, tag="j38")
                    pos1 = work.tile([128, 1], f32, tag="pos1")
                    nc.vector.scalar_tensor_tensor(
                        out=j38, in0=iota38, scalar=jself[:, t:t + 1],
                        in1=hsum, op0=ALU.is_equal, op1=ALU.mult,
                        accum_out=pos1)
                    j38b = work.tile([128, J], f32, tag="j38b")
                    poscos = work.tile([128, 1], f32, tag="poscos")
                    nc.vector.scalar_tensor_tensor(
                        out=j38b, in0=iota38, scalar=jself[:, t:t + 1],
                        in1=hcos[:, t * J:(t + 1) * J], op0=ALU.is_equal,
                        op1=ALU.mult, accum_out=poscos)
                    rD = work.tile([128, 1], f32, tag="rD")
                    nc.vector.reciprocal(out=rD, in_=D)
                    lnD = work.tile([128, 1], f32, tag="lnD")
                    nc.scalar.activation(out=lnD, in_=D, func=AF.Ln)
                    # term = S*lnD + pos1/D - poscos/(fn*temp)
                    ta = work.tile([128, 1], f32, tag="ta")
                    nc.vector.tensor_mul(out=ta, in0=pos1, in1=rD)
                    tb = work.tile([128, 1], f32, tag="tb")
                    nc.vector.scalar_tensor_tensor(
                        out=tb, in0=lnD, scalar=float(S), in1=ta,
                        op0=ALU.mult, op1=ALU.add)
                    tcm = work.tile([128, 1], f32, tag="tcm")
                    nc.vector.tensor_mul(out=tcm, in0=poscos, in1=s_col)
                    term = work.tile([128, 1], f32, tag="term")
                    nc.vector.tensor_sub(out=term, in0=tb, in1=tcm)
                    # one-hot class accumulation (masked)
                    oh = work.tile([128, C], f32, tag="oh")
                    nc.vector.tensor_scalar(
                        out=oh, in0=iota38[:, :C], scalar1=labf[:, t:t + 1],
                        scalar2=mskf[:, t:t + 1], op0=ALU.is_equal,
                        op1=ALU.mult)
                    nc.vector.tensor_add(out=acc_n, in0=acc_n, in1=oh)
                    oht = work.tile([128, C], f32, tag="oht")
                    nc.vector.tensor_scalar(
                        out=oht, in0=oh, scalar1=term, scalar2=None,
                        op0=ALU.mult)
                    nc.vector.tensor_add(out=acc_c, in0=acc_c, in1=oht)

            # ---- finalize: partition-reduce the per-class accumulators ----
            stage = persist.tile([1, 2 * C], f32, tag="stage")
            with tc.tile_pool(name="psum_out", bufs=2, space="PSUM") as psum_o:
                po = psum_o.tile([1, C], f32, tag="po")
                nc.tensor.matmul(po, ones_col, acc_c, start=True, stop=True)
                nc.scalar.copy(out=stage[0:1, :C], in_=po)
                po2 = psum_o.tile([1, C], f32, tag="po2")
                nc.tensor.matmul(po2, ones_col, acc_n, start=True, stop=True)
                nc.scalar.copy(out=stage[0:1, C:], in_=po2)
            nc.sync.dma_start(out=out_d.rearrange("a b -> (a b)")[None, :],
                              in_=stage)

    nc.finalize()
    return nc


_CACHE = {}


def get_program(P):
    if P not in _CACHE:
        _CACHE[P] = build(P)
    return _CACHE[P]


def prepare_inputs(memory_bank, pred_rep, labels, mask, which_memory):
    """Host-side sharding: compact masked pixels, pad, split across cores."""
    memory_bank = np.asarray(memory_bank, dtype=np.float32)
    pred_rep = np.asarray(pred_rep, dtype=np.float32)
    lab = np.asarray(labels).reshape(-1).astype(np.int64)
    msk = np.asarray(mask).reshape(-1).astype(bool)
    wm = np.asarray(which_memory).reshape(-1).astype(np.int64)

    memT = np.ascontiguousarray(
        memory_bank.reshape(M, F).T).astype(ml_dtypes.bfloat16)

    featsT = np.ascontiguousarray(
        pred_rep.transpose(1, 0, 2, 3).reshape(F, -1))

    sel = np.flatnonzero(msk)
    n_sel = len(sel)
    unit = N_CORES * 128
    P_tot = max(((n_sel + unit - 1) // unit) * unit, unit)
    P = P_tot // N_CORES
    T = P // 128

    f_pad = np.ones((F, P_tot), np.float32)
    f_pad[:, :n_sel] = featsT[:, sel]
    lab_pad = np.zeros(P_tot, np.float32)
    lab_pad[:n_sel] = lab[sel]
    jsel_pad = np.zeros(P_tot, np.float32)
    jsel_pad[:n_sel] = 2 * lab[sel] + (1 - wm[sel])
    msk_pad = np.zeros(P_tot, np.float32)
    msk_pad[:n_sel] = 1.0

    in_maps = []
    for i in range(N_CORES):
        cs = slice(i * P, (i + 1) * P)
        in_maps.append({
            "feats": np.ascontiguousarray(f_pad[:, cs]),
            "memT": memT,
            "labf": np.ascontiguousarray(lab_pad[cs].reshape(T, 128).T),
            "jself": np.ascontiguousarray(jsel_pad[cs].reshape(T, 128).T),
            "mskf": np.ascontiguousarray(msk_pad[cs].reshape(T, 128).T),
        })
    return P, in_maps


def finalize(outs, num_classes):
    agg = np.zeros((2, C), np.float64)
    for o in outs:
        agg += np.asarray(o, dtype=np.float64)
    contrib, cnt = agg[0], agg[1]
    nz = cnt > 0.5
    per_class = np.where(nz, contrib / (np.maximum(cnt, 1.0) * S), 0.0)
    loss = per_class[:num_classes].sum() / max(int(nz[:num_classes].sum()), 1)
    return np.float32(loss)


def kernel(memory_bank, pred_rep, labels, mask, which_memory, num_classes,
           temp=0.5):
    assert int(num_classes) == C and abs(temp - TEMP) < 1e-12
    P, in_maps = prepare_inputs(memory_bank, pred_rep, labels, mask,
                                which_memory)
    nc = get_program(P)
    res = run_bass_kernel_spmd(nc, in_maps, core_ids=list(range(N_CORES)))
    outs = [res.results[i]["out"] for i in range(N_CORES)]
    return finalize(outs, int(num_classes))
